# revision 1
# baseline (speedup 1.0000x reference)
"""Self-contained distributed kernel for nn_Attention_62543313764936.

LayerNorm -> QKV projection -> (torch-.view style) 8-head attention over
w-windows -> output projection, for x of shape [B=4, C=16, D=16, W=32, DM=512].

Sharding: data-parallel over the flattened (B, C) axis (64 units -> 8 per
NeuronCore). The reference's head reshape carves the head axis out of the
flattened (C, D, W, feature) axes; algebraically the attention decomposes into
independent 32x32 attentions over groups of 4 consecutive tokens (all within
one (b, c, d) row), with q/k/v taken from contiguous 192-wide slices of the
token's 1536-wide QKV row.  Concretely, for qkv laid out [N_tok, 1536]
(token-major, row-major), the reshaped attention operand is exactly
qkv.reshape(N_tok*8, 192) with consecutive 32-row blocks forming attention
groups (q = cols 0:64, k = 64:128, v = 128:192), and the attention output
[N_tok*8, 64] viewed as [N_tok, 512] is the vhat fed to the output
projection.  Because groups are 4-token aligned, any contiguous token shard in
multiples of 4 tokens is fully local -> pure data parallelism, weights
replicated, no collectives.
"""

import numpy as np
import jax
import jax.numpy as jnp

B, C, D, W, DM = 4, 16, 16, 32, 512
N_CORES = 8
LN_EPS = 1e-5

_TOK_PER_CORE = (B * C // N_CORES) * D * W  # 8 units * 512 tok = 4096


def _local_compute(x, ln_gamma, ln_beta, W_qkv, W_out, b_out):
    # x: [N_tok, DM] shard on one core
    mean = jnp.mean(x, axis=-1, keepdims=True)
    var = jnp.mean(jnp.square(x - mean), axis=-1, keepdims=True)
    xn = (x - mean) * jax.lax.rsqrt(var + LN_EPS) * ln_gamma + ln_beta

    qkv = xn @ W_qkv                       # [N_tok, 1536]
    r = qkv.reshape(-1, 32, 192)           # [n_groups, 32, 192]
    q = r[:, :, 0:64]
    k = r[:, :, 64:128]
    v = r[:, :, 128:192]

    s = jnp.einsum("gwe,gve->gwv", q, k) * (64.0 ** 0.5)
    p = jax.nn.softmax(s, axis=-1)
    o = jnp.einsum("gwv,gve->gwe", p, v)   # [n_groups, 32, 64]

    vhat = o.reshape(-1, DM)               # [N_tok, DM]
    return vhat @ W_out + b_out


_pmapped = None


def _get_pmapped():
    global _pmapped
    if _pmapped is None:
        devs = jax.devices()[:N_CORES]
        _pmapped = jax.pmap(
            _local_compute,
            in_axes=(0, None, None, None, None, None),
            devices=devs,
        )
    return _pmapped


def kernel(x, ln_gamma, ln_beta, W_qkv, W_out, b_out):
    x = np.asarray(x, dtype=np.float32)
    # [B,C,D,W,DM] -> contiguous token-major shards [8, 4096, DM]
    xs = np.ascontiguousarray(x.reshape(N_CORES, _TOK_PER_CORE, DM))
    fn = _get_pmapped()
    out = fn(
        jnp.asarray(xs),
        jnp.asarray(ln_gamma, dtype=jnp.float32),
        jnp.asarray(ln_beta, dtype=jnp.float32),
        jnp.asarray(W_qkv, dtype=jnp.float32),
        jnp.asarray(W_out, dtype=jnp.float32),
        jnp.asarray(b_out, dtype=jnp.float32),
    )
    out = np.asarray(out, dtype=np.float32).reshape(B, C, D, W, DM)
    return out

